# revision 24
# baseline (speedup 1.0000x reference)
"""Expert-parallel Switch-Transformer MoE layer for 8 Trainium2 NeuronCores.

Strategy (SPMD, one program, per-core inputs):
  - Token-parallel router, 1-pass bf16 fc1 (xh@wh only). Tokens whose top-2
    logit gap < TH get their logits recomputed with the exact 3-term bf16
    split (xh@wh + xh@wl + xl@wh ~ fp32); that keeps argmax bit-exact vs the
    fp32 reference while paying the 3-pass cost for only ~256 of 2048 tokens.
    Gate error from the 1-pass softmax is <=5e-3 relative (measured), well
    inside the 2e-2 budget, so gates are not corrected.
  - Single-matmul cumsum over all 16 token tiles + strided-prefix matmul
    (SPM) gives per-shard positions; AllGather of per-shard expert counts
    gives global first-come positions.
  - Slot table laid out [E, 2304] (capacity 2048 + 256 pad per expert) so a
    single ReduceScatter(add) both merges the 8 cores' scatter tables and
    delivers each core exactly its expert's slice. (AllToAll measured 264us
    for 64KB on this runtime; AG+select 108us; RS is ~15us.)
  - Expert-parallel FFN (bf16): core k holds expert k's weights (preloaded
    at t=0); tokens fetched by transpose-dma_gather from replicated bf16 x.
    512-token compute chunks; AllGather of expert outputs in 256-row pieces
    overlaps following chunks' compute.
  - Combine: each core gathers its own tokens' rows by slot, applies its
    locally-known gate, writes its token shard. Host concatenates.
"""
import sys

for _p in ("/opt/trn_rl_repo", "/root/.axon_site/_ro/trn_rl_repo"):
    if _p not in sys.path:
        sys.path.append(_p)

import numpy as np

import concourse.bacc as bacc
import concourse.bass as bass
import concourse.mybir as mybir
import concourse.tile as tile
from concourse import bass_utils

F32 = mybir.dt.float32
BF16 = mybir.dt.bfloat16
I16 = mybir.dt.int16
I32 = mybir.dt.int32
U32 = mybir.dt.uint32
Alu = mybir.AluOpType
Act = mybir.ActivationFunctionType
X = mybir.AxisListType.X

T, D, E, H = 16384, 1024, 8, 4096
NC = 8
TS = T // NC                # tokens per shard = 2048
C = T // E                  # expert capacity = 2048
NTT = TS // 128             # 16 token tiles per shard
NTC = TS // 512             # 4 router token chunks
CHUNK = 256                 # FFN compute chunk
NCH = C // CHUNK            # 8 compute chunks
AGCH = 256                  # combine AllGather chunk (rec formula unit)
NAG = C // AGCH             # 8 AGs
NJT = H // 128              # 32
SENT = T                    # sentinel row id -> zero pad row
TPAD = C + AGCH             # per-expert RS block = 2304
R = 256                     # router recompute slots (max ambig/core ~139)
TH = 0.03                   # ambiguity threshold (4x measured 1-pass err)
RG = [list(range(NC))]

DEBUG_OUTPUTS = False


def build(debug_outputs=DEBUG_OUTPUTS):
    nc = bacc.Bacc("TRN2", target_bir_lowering=False, debug=False, num_devices=NC)

    # --------- per-core inputs (host pre-arranged, straight DMA loads) ---------
    xTh = nc.dram_tensor("xTh", [NTC, 128, E, 512], BF16, kind="ExternalInput")
    wr1h = nc.dram_tensor("wr1h", [128, E, D], BF16, kind="ExternalInput")
    wr1l = nc.dram_tensor("wr1l", [128, E, D], BF16, kind="ExternalInput")
    wr2 = nc.dram_tensor("wr2", [128, E, E], F32, kind="ExternalInput")
    br1 = nc.dram_tensor("br1", [128, E], F32, kind="ExternalInput")
    br2 = nc.dram_tensor("br2", [1, E], F32, kind="ExternalInput")
    xbf = nc.dram_tensor("xbf", [T + 1, D], BF16, kind="ExternalInput")
    xbl = nc.dram_tensor("xbl", [T + 1, D], BF16, kind="ExternalInput")
    w1 = nc.dram_tensor("w1", [128, E, H], BF16, kind="ExternalInput")
    w2 = nc.dram_tensor("w2", [128, NJT, D], BF16, kind="ExternalInput")
    b1 = nc.dram_tensor("b1", [128, NJT], F32, kind="ExternalInput")
    b2 = nc.dram_tensor("b2", [1, D], F32, kind="ExternalInput")
    UT = nc.dram_tensor("UT", [128, 128], F32, kind="ExternalInput")
    SPM = nc.dram_tensor("SPM", [128, 128], F32, kind="ExternalInput")
    IOTA8 = nc.dram_tensor("IOTA8", [128, E], F32, kind="ExternalInput")
    TOKID = nc.dram_tensor("TOKID", [128, NTT], F32, kind="ExternalInput")
    KOFF = nc.dram_tensor("KOFF", [1, 1], F32, kind="ExternalInput")
    MASK = nc.dram_tensor("MASK", [E, 128], F32, kind="ExternalInput")
    IDN = nc.dram_tensor("IDN", [128, 128], F32, kind="ExternalInput")

    out = nc.dram_tensor("out", [TS, D], F32, kind="ExternalOutput")
    if debug_outputs:
        dbg_slot = nc.dram_tensor("dbg_slot", [128, NTT], F32, kind="ExternalOutput")
        dbg_gate = nc.dram_tensor("dbg_gate", [128, NTT], F32, kind="ExternalOutput")
        dbg_eid = nc.dram_tensor("dbg_eid", [128, NTT], F32, kind="ExternalOutput")
        dbg_rec = nc.dram_tensor("dbg_rec", [128, NTT], F32, kind="ExternalOutput")
        dbg_amb = nc.dram_tensor("dbg_amb", [128, NTT], F32, kind="ExternalOutput")
        dbg_cnt = nc.dram_tensor("dbg_cnt", [E, E], F32, kind="ExternalOutput")
        dbg_tblidx = nc.dram_tensor("dbg_tblidx", [128, NTT], F32,
                                    kind="ExternalOutput")

    with tile.TileContext(nc) as tc:
        with tc.tile_pool(name="fw", bufs=1) as fw, \
             tc.tile_pool(name="sbs", bufs=1) as sbs, \
             tc.tile_pool(name="sbt", bufs=2) as sbt, \
             tc.tile_pool(name="psb", bufs=4, space="PSUM") as psb, \
             tc.tile_pool(name="pss", bufs=2, space="PSUM") as pss, \
             tc.tile_pool(name="dram", bufs=1, space="DRAM") as dram:

            # ---------- FFN w1: issue load first so it runs at t=0 ----
            # (w2 loads after the router frees its SBUF; its 22us DMA hides
            # behind the table ReduceScatter + first chunk's layer-1.)
            w1_sb = fw.tile([128, E, H], BF16)
            nc.sync.dma_start(w1_sb[:], w1[:])

            # ---------- persistent small tiles ----------
            br2_row = sbs.tile([1, E], F32)
            nc.sync.dma_start(br2_row[:], br2[:])
            br2_rep = sbs.tile([128, E], F32)
            nc.gpsimd.partition_broadcast(br2_rep[:], br2_row[:])
            b1_sb = sbs.tile([128, NJT], F32)
            nc.sync.dma_start(b1_sb[:], b1[:])
            b2_row = sbs.tile([1, D], F32)
            nc.sync.dma_start(b2_row[:], b2[:])
            b2_rep = sbs.tile([128, D], F32)
            nc.gpsimd.partition_broadcast(b2_rep[:], b2_row[:])
            ut_sb = sbs.tile([128, 128], F32)
            nc.sync.dma_start(ut_sb[:], UT[:])
            spm_sb = sbs.tile([128, 128], F32)
            nc.sync.dma_start(spm_sb[:], SPM[:])
            ones_sb = sbs.tile([128, 128], F32)
            nc.vector.memset(ones_sb[:], 1.0)
            iota_sb = sbs.tile([128, E], F32)
            nc.sync.dma_start(iota_sb[:], IOTA8[:])
            tokid_sb = sbs.tile([128, NTT], F32)
            nc.sync.dma_start(tokid_sb[:], TOKID[:])
            koff_row = sbs.tile([1, 1], F32)
            nc.sync.dma_start(koff_row[:], KOFF[:])
            koff_rep = sbs.tile([128, 1], F32)
            nc.gpsimd.partition_broadcast(koff_rep[:], koff_row[:])
            mask_sb = sbs.tile([E, 128], F32)
            nc.sync.dma_start(mask_sb[:], MASK[:])
            idn_sb = sbs.tile([128, 128], F32)
            nc.sync.dma_start(idn_sb[:], IDN[:])

            gate_all = sbs.tile([128, NTT], F32)
            gidx_rep = sbs.tile([128, 128], I16)     # dispatch gather idx
            ridx_rep = sbs.tile([128, TS // 16], I16)  # recon gather idx
            zbf = sbs.tile([1, 256], BF16)
            nc.vector.memset(zbf[:], 0.0)

            # ---------- DRAM scratch ----------
            cnt_in_d = dram.tile([1, E], F32)
            cnt_all_d = dram.tile([E, E], F32, addr_space="Shared")
            table_d = dram.tile([E * TPAD], F32)     # slot->tok+1, [E,2304] blocks
            rstbl_d = dram.tile([TPAD], F32)         # RS output: my expert's block
            eid_d = dram.tile([TS + 128], F32)       # eid merge bounce
            atbl_d = dram.tile([R + 128], F32)       # ambig compaction table
            ridx_f_d = dram.tile([TS], F32)          # recon idx bounce
            agin_d = dram.tile([C, D], BF16)
            oe_all_d = dram.tile([T + 1, D], BF16)

            for zi in range(4):
                nc.sync.dma_start(oe_all_d[T:T + 1, zi * 256:(zi + 1) * 256],
                                  zbf[:])

            # ===================== ROUTER + DISPATCH PREP =====================
            with tc.tile_pool(name="rp", bufs=1) as rp:
                eid_all = rp.tile([128, NTT], F32)
                amb_all = rp.tile([128, NTT], F32)

                with tc.tile_pool(name="rt", bufs=1) as rt:
                    wr1h_sb = rt.tile([128, E, D], BF16)
                    nc.sync.dma_start(wr1h_sb[:], wr1h[:])
                    wr1l_sb = rt.tile([128, E, D], BF16)
                    nc.sync.dma_start(wr1l_sb[:], wr1l[:])
                    wr2_sb = rt.tile([128, E, E], F32)
                    nc.sync.dma_start(wr2_sb[:], wr2[:])
                    br1_sb = rt.tile([128, E], F32)
                    nc.sync.dma_start(br1_sb[:], br1[:])

                    # ---- 1-pass bf16 fc1 + fused per-chunk logits ----
                    with tc.tile_pool(name="rx", bufs=2) as rx:
                        for tcn in range(NTC):
                            xh_c = rx.tile([128, E, 512], BF16, tag="xh")
                            nc.sync.dma_start(xh_c[:], xTh[tcn])
                            ht = rx.tile([128, E, 512], F32, tag="ht", bufs=1)
                            for jt in range(E):
                                js = slice(jt * 128, (jt + 1) * 128)
                                ps = psb.tile([128, 512], F32, tag="pbig")
                                for dt in range(E):
                                    nc.tensor.matmul(
                                        ps[:], wr1h_sb[:, dt, js], xh_c[:, dt, :],
                                        start=(dt == 0), stop=(dt == E - 1))
                                nc.scalar.activation(
                                    ht[:, jt, :], ps[:],
                                    Act.Relu, bias=br1_sb[:, jt:jt + 1], scale=1.0)

                            for u in range(4):
                                tt = tcn * 4 + u
                                us = slice(u * 128, (u + 1) * 128)
                                ps2 = pss.tile([128, E], F32, tag="psmall")
                                for jt in range(E):
                                    nc.tensor.matmul(
                                        ps2[:], ht[:, jt, us], wr2_sb[:, jt, :],
                                        start=(jt == 0), stop=(jt == E - 1))
                                lg = sbt.tile([128, E], F32, tag="lg")
                                nc.vector.tensor_tensor(lg[:], ps2[:], br2_rep[:],
                                                        Alu.add)
                                mx = sbt.tile([128, E], F32, tag="mx")
                                mi = sbt.tile([128, E], U32, tag="mi")
                                nc.vector.max_with_indices(mx[:], mi[:], lg[:])
                                nc.vector.tensor_copy(eid_all[:, tt:tt + 1],
                                                      mi[:, 0:1])
                                negmx = sbt.tile([128, 1], F32, tag="negmx")
                                nc.vector.tensor_scalar(negmx[:], mx[:, 0:1], -1.0,
                                                        None, Alu.mult)
                                ex = sbt.tile([128, E], F32, tag="ex")
                                nc.scalar.activation(ex[:], lg[:], Act.Exp,
                                                     bias=negmx[:], scale=1.0)
                                sm = sbt.tile([128, 1], F32, tag="sm")
                                nc.vector.reduce_sum(sm[:], ex[:], axis=X)
                                nc.vector.reciprocal(gate_all[:, tt:tt + 1], sm[:])
                                # top-2 gap -> ambiguity flag
                                oh1 = sbt.tile([128, E], F32, tag="oh1")
                                nc.vector.tensor_scalar(
                                    oh1[:], iota_sb[:], eid_all[:, tt:tt + 1],
                                    None, Alu.is_equal)
                                nc.vector.tensor_scalar(oh1[:], oh1[:], 1e9, None,
                                                        Alu.mult)
                                lgm = sbt.tile([128, E], F32, tag="lgm")
                                nc.vector.tensor_tensor(lgm[:], lg[:], oh1[:],
                                                        Alu.subtract)
                                mx2 = sbt.tile([128, 1], F32, tag="mx2")
                                nc.vector.reduce_max(mx2[:], lgm[:], axis=X)
                                gap = sbt.tile([128, 1], F32, tag="gap")
                                nc.vector.tensor_tensor(gap[:], mx[:, 0:1], mx2[:],
                                                        Alu.subtract)
                                nc.vector.tensor_scalar(
                                    amb_all[:, tt:tt + 1], gap[:], TH, None,
                                    Alu.is_lt)

                    # ---- compact ambiguous tokens into R recompute slots ----
                    cum_a = pss.tile([128, NTT], F32, tag="pc16", bufs=1)
                    nc.tensor.matmul(cum_a[:], ut_sb[:], amb_all[:],
                                     start=True, stop=True)
                    tot_a = pss.tile([128, NTT], F32, tag="ptr", bufs=1)
                    nc.tensor.matmul(tot_a[:], ones_sb[:], amb_all[:],
                                     start=True, stop=True)
                    colcnt = sbt.tile([1, NTT], F32, tag="colcnt")
                    nc.vector.tensor_copy(colcnt[:], tot_a[0:1, :])
                    ct_ps = pss.tile([NTT, 1], F32, tag="ptr", bufs=1)
                    nc.tensor.transpose(ct_ps[:], colcnt[:], idn_sb[0:1, 0:1])
                    colT = sbt.tile([NTT, 1], F32, tag="colT")
                    nc.vector.tensor_copy(colT[:], ct_ps[:])
                    ip_ps = pss.tile([NTT, 1], F32, tag="ptr", bufs=1)
                    nc.tensor.matmul(ip_ps[:], ut_sb[0:NTT, 0:NTT], colT[:],
                                     start=True, stop=True)
                    excl = sbt.tile([NTT, 1], F32, tag="excl")
                    nc.vector.tensor_tensor(excl[:], ip_ps[:], colT[:],
                                            Alu.subtract)
                    er_ps = pss.tile([1, NTT], F32, tag="ptr", bufs=1)
                    nc.tensor.transpose(er_ps[:], excl[:], idn_sb[0:NTT, 0:NTT])
                    excl_row = sbt.tile([1, NTT], F32, tag="exclrow")
                    nc.vector.tensor_copy(excl_row[:], er_ps[:])
                    excl_rep = sbt.tile([128, NTT], F32, tag="exclrep")
                    nc.gpsimd.partition_broadcast(excl_rep[:], excl_row[:])

                    asl = sbt.tile([128, NTT], F32, tag="asl")
                    nc.vector.tensor_tensor(asl[:], cum_a[:], excl_rep[:], Alu.add)
                    nc.vector.tensor_scalar(asl[:], asl[:], float(1 + R), None,
                                            Alu.subtract)
                    nc.vector.tensor_tensor(asl[:], asl[:], amb_all[:], Alu.mult)
                    nc.vector.tensor_scalar(asl[:], asl[:], float(R), None,
                                            Alu.add)
                    nc.vector.tensor_scalar(asl[:], asl[:], float(R), None,
                                            Alu.min)
                    aidx = sbt.tile([128, NTT], I32, tag="aidx")
                    nc.vector.tensor_copy(aidx[:], asl[:])

                    za = sbt.tile([128, (R + 128) // 128], F32, tag="za")
                    nc.vector.memset(za[:], 0.0)
                    nc.sync.dma_start(
                        atbl_d[:].rearrange("(p n) -> p n", p=128), za[:])
                    atbl2d = atbl_d[:].rearrange("(n e) -> n e", e=1)
                    for i in range(NTT):
                        nc.gpsimd.indirect_dma_start(
                            atbl2d,
                            bass.IndirectOffsetOnAxis(ap=aidx[:, i:i + 1], axis=0),
                            tokid_sb[:, i:i + 1], None)

                    # readback: gather idx (16-wrap) + merge offsets (128-wrap)
                    at16 = sbt.tile([16, R // 16], F32, tag="at16")
                    nc.sync.dma_start(
                        at16[:], atbl_d[0:R].rearrange("(n p) -> p n", p=16))
                    ag16 = sbt.tile([16, R // 16], F32, tag="ag16")
                    nc.vector.tensor_scalar(ag16[:], at16[:], 0.0,
                                            float(SENT + 1), Alu.is_equal,
                                            Alu.mult)
                    nc.vector.tensor_tensor(ag16[:], ag16[:], at16[:], Alu.add)
                    nc.vector.tensor_scalar(ag16[:], ag16[:], 1.0, None,
                                            Alu.subtract)
                    gidx_a = sbt.tile([128, R // 16], I16, tag="gidxa")
                    nc.vector.tensor_copy(gidx_a[0:16, :], ag16[:])
                    for rep in range(1, 8):
                        nc.sync.dma_start(gidx_a[16 * rep:16 * (rep + 1), :],
                                          gidx_a[0:16, :])

                    # gather ambiguous tokens' x (hi+lo), 3-pass fc1, logits
                    with tc.tile_pool(name="rc", bufs=1) as rc:
                        axh = rc.tile([128, E, R], BF16)
                        nc.gpsimd.dma_gather(axh[:], xbf[:], gidx_a[:], R, R, D,
                                             transpose=True)
                        axl = rc.tile([128, E, R], BF16)
                        nc.gpsimd.dma_gather(axl[:], xbl[:], gidx_a[:], R, R, D,
                                             transpose=True)
                        htr = rc.tile([128, E, R], F32)
                        for jt in range(E):
                            js = slice(jt * 128, (jt + 1) * 128)
                            ps = psb.tile([128, R], F32, tag="pbig")
                            first = True
                            for dt in range(E):
                                for wop, xop in ((wr1h_sb, axh), (wr1l_sb, axh),
                                                 (wr1h_sb, axl)):
                                    nc.tensor.matmul(
                                        ps[:], wop[:, dt, js], xop[:, dt, :],
                                        start=first,
                                        stop=(dt == E - 1 and xop is axl))
                                    first = False
                            nc.scalar.activation(
                                htr[:, jt, :], ps[:], Act.Relu,
                                bias=br1_sb[:, jt:jt + 1], scale=1.0)

                        eidr = sbt.tile([128, R // 128], F32, tag="eidr")
                        for u in range(R // 128):
                            us = slice(u * 128, (u + 1) * 128)
                            ps2 = pss.tile([128, E], F32, tag="psmall")
                            for jt in range(E):
                                nc.tensor.matmul(
                                    ps2[:], htr[:, jt, us], wr2_sb[:, jt, :],
                                    start=(jt == 0), stop=(jt == E - 1))
                            lgr = sbt.tile([128, E], F32, tag="lgr")
                            nc.vector.tensor_tensor(lgr[:], ps2[:], br2_rep[:],
                                                    Alu.add)
                            mxr = sbt.tile([128, E], F32, tag="mxr")
                            mir = sbt.tile([128, E], U32, tag="mir")
                            nc.vector.max_with_indices(mxr[:], mir[:], lgr[:])
                            nc.vector.tensor_copy(eidr[:, u:u + 1], mir[:, 0:1])

                    # merge corrected eids via DRAM bounce
                    nc.sync.dma_start(
                        eid_d[0:TS].rearrange("(n p) -> p n", p=128), eid_all[:])
                    at128 = sbt.tile([128, R // 128], F32, tag="at128")
                    nc.sync.dma_start(
                        at128[:], atbl_d[0:R].rearrange("(n p) -> p n", p=128))
                    offs = sbt.tile([128, R // 128], F32, tag="offs")
                    nc.vector.tensor_scalar(offs[:], at128[:], 1.0, None,
                                            Alu.subtract)
                    nc.vector.tensor_scalar(offs[:], offs[:], koff_rep[:, 0:1],
                                            None, Alu.subtract)
                    neg = sbt.tile([128, R // 128], F32, tag="neg")
                    nc.vector.tensor_scalar(neg[:], offs[:], 0.0, None, Alu.is_lt)
                    tmu = sbt.tile([128, R // 128], F32, tag="tmu")
                    nc.vector.tensor_tensor(tmu[:], offs[:], neg[:], Alu.mult)
                    nc.vector.tensor_tensor(offs[:], offs[:], tmu[:], Alu.subtract)
                    nc.vector.tensor_scalar(tmu[:], neg[:], float(TS), None,
                                            Alu.mult)
                    nc.vector.tensor_tensor(offs[:], offs[:], tmu[:], Alu.add)
                    oi = sbt.tile([128, R // 128], I32, tag="oi")
                    nc.vector.tensor_copy(oi[:], offs[:])
                    eid2d = eid_d[:].rearrange("(n e) -> n e", e=1)
                    for u in range(R // 128):
                        nc.gpsimd.indirect_dma_start(
                            eid2d,
                            bass.IndirectOffsetOnAxis(ap=oi[:, u:u + 1], axis=0),
                            eidr[:, u:u + 1], None)
                    eidm = rp.tile([128, NTT], F32)
                    nc.sync.dma_start(
                        eidm[:], eid_d[0:TS].rearrange("(n p) -> p n", p=128))

                    # ---- onehots, counts, positions (batched) ----
                    oh_all = rp.tile([128, NTT, E], F32)
                    oh_flat = oh_all[:].rearrange("p t e -> p (t e)")
                    for tt in range(NTT):
                        nc.vector.tensor_scalar(
                            oh_all[:, tt, :], iota_sb[:], eidm[:, tt:tt + 1],
                            None, Alu.is_equal)

                    cum_ps = psb.tile([128, NTT * E], F32, tag="pbig")
                    nc.tensor.matmul(cum_ps[:], ut_sb[:], oh_flat,
                                     start=True, stop=True)
                    cum_sb = rp.tile([128, NTT * E], F32)
                    nc.vector.tensor_copy(cum_sb[:], cum_ps[:])
                    tot_ps = psb.tile([128, NTT * E], F32, tag="pbig")
                    nc.tensor.matmul(tot_ps[:], ones_sb[:], oh_flat,
                                     start=True, stop=True)
                    tot_sb = sbt.tile([128, NTT * E], F32, tag="w128")
                    nc.vector.tensor_copy(tot_sb[:], tot_ps[:])

                    # expert counts [1, E] = sum over tiles of totals
                    cnt_row = sbt.tile([1, E], F32, tag="cntrow")
                    nc.vector.tensor_reduce(
                        cnt_row[:].unsqueeze(2),
                        tot_sb[0:1, :].rearrange("a (t e) -> a e t", e=E),
                        X, Alu.add)
                    nc.sync.dma_start(cnt_in_d[:], cnt_row[:])
                    nc.gpsimd.collective_compute(
                        "AllGather", Alu.bypass, replica_groups=RG,
                        ins=[cnt_in_d[:]], outs=[cnt_all_d[:]])

                    # cross-tile exclusive prefix via SPM matmul (transposed)
                    totT_ps = psb.tile([128, 128], F32, tag="pbig")
                    nc.tensor.transpose(totT_ps[:], tot_sb[:], idn_sb[:])
                    totT_sb = sbt.tile([128, 128], F32, tag="w128")
                    nc.vector.tensor_copy(totT_sb[:], totT_ps[:])
                    pfxT_ps = psb.tile([128, 128], F32, tag="pbig")
                    nc.tensor.matmul(pfxT_ps[:], spm_sb[:], totT_sb[:],
                                     start=True, stop=True)
                    pfxT_sb = sbt.tile([128, 128], F32, tag="w128")
                    nc.vector.tensor_copy(pfxT_sb[:], pfxT_ps[:])
                    pfx_ps = psb.tile([128, 128], F32, tag="pbig")
                    nc.tensor.transpose(pfx_ps[:], pfxT_sb[:], idn_sb[:])
                    pfx_sb = sbt.tile([128, 128], F32, tag="w128")
                    nc.vector.tensor_copy(pfx_sb[:], pfx_ps[:])

                    # cross-core base offsets
                    cnt_sb = sbt.tile([E, E], F32, tag="cntsb")
                    nc.sync.dma_start(cnt_sb[:], cnt_all_d[:])
                    baseps = pss.tile([128, E], F32, tag="psmall")
                    nc.tensor.matmul(baseps[:], mask_sb[:], cnt_sb[:],
                                     start=True, stop=True)
                    base_rep = sbt.tile([128, E], F32, tag="baserep")
                    nc.vector.tensor_copy(base_rep[:], baseps[:])

                    if debug_outputs:
                        nc.sync.dma_start(dbg_eid[:], eidm[:])
                        nc.sync.dma_start(dbg_gate[:], gate_all[:])
                        nc.sync.dma_start(dbg_amb[:], amb_all[:])
                        nc.sync.dma_start(dbg_cnt[:], cnt_sb[:])

                    # pos (inclusive) = (cum + prefix + base) . onehot, batched
                    t0 = sbt.tile([128, NTT, E], F32, tag="w128")
                    t0f = t0[:].rearrange("p t e -> p (t e)")
                    nc.vector.tensor_tensor(t0f, cum_sb[:], pfx_sb[:], Alu.add)
                    base_b = base_rep[:].unsqueeze(1).to_broadcast([128, NTT, E])
                    nc.vector.tensor_tensor(t0[:], t0[:], base_b, Alu.add)
                    nc.vector.tensor_tensor(t0f, t0f, oh_flat, Alu.mult)
                    pos_all = rp.tile([128, NTT], F32)
                    nc.vector.tensor_reduce(pos_all[:].unsqueeze(2), t0[:],
                                            X, Alu.add)
                    nc.vector.tensor_scalar(pos_all[:], pos_all[:], 1.0, None,
                                            Alu.subtract)
                    valid_all = sbt.tile([128, NTT], F32, tag="validall")
                    nc.vector.tensor_scalar(valid_all[:], pos_all[:], float(C),
                                            None, Alu.is_lt)
                    pi = sbt.tile([128, NTT], I32, tag="pi")
                    nc.vector.tensor_copy(pi[:], pos_all[:])
                    lo = sbt.tile([128, NTT], I32, tag="lo")
                    nc.vector.tensor_scalar(lo[:], pi[:], AGCH - 1, None,
                                            Alu.bitwise_and)
                    hi = sbt.tile([128, NTT], I32, tag="hi")
                    nc.vector.tensor_scalar(hi[:], pi[:], 8, None,
                                            Alu.arith_shift_right)
                    lof = sbt.tile([128, NTT], F32, tag="lof")
                    nc.vector.tensor_copy(lof[:], lo[:])
                    hif = sbt.tile([128, NTT], F32, tag="hif")
                    nc.vector.tensor_copy(hif[:], hi[:])

                    # slot = eid*TPAD + (valid ? pos : 2048 + lo)
                    off = sbt.tile([128, NTT], F32, tag="off")
                    nc.vector.tensor_tensor(off[:], pos_all[:], lof[:],
                                            Alu.subtract)
                    nc.vector.tensor_scalar(off[:], off[:], float(C), None,
                                            Alu.subtract)
                    nc.vector.tensor_tensor(off[:], off[:], valid_all[:],
                                            Alu.mult)
                    nc.vector.tensor_tensor(off[:], off[:], lof[:], Alu.add)
                    nc.vector.tensor_scalar(off[:], off[:], float(C), None,
                                            Alu.add)
                    slot_all = sbt.tile([128, NTT], F32, tag="slotall")
                    nc.vector.tensor_scalar(slot_all[:], eidm[:], float(TPAD),
                                            None, Alu.mult)
                    nc.vector.tensor_tensor(slot_all[:], slot_all[:], off[:],
                                            Alu.add)
                    slotidx = sbt.tile([128, NTT], I32, tag="slotidx")
                    nc.vector.tensor_copy(slotidx[:], slot_all[:])

                    # recon row = (pos>>8)*2048 + eid*256 + (pos&255), or SENT
                    rec_all = rp.tile([128, NTT], F32)
                    rr = sbt.tile([128, NTT], F32, tag="rr")
                    nc.vector.tensor_scalar(rr[:], hif[:], float(AGCH * NC),
                                            None, Alu.mult)
                    r2 = sbt.tile([128, NTT], F32, tag="r2")
                    nc.vector.tensor_scalar(r2[:], eidm[:], float(AGCH), None,
                                            Alu.mult)
                    nc.vector.tensor_tensor(rr[:], rr[:], r2[:], Alu.add)
                    nc.vector.tensor_tensor(rr[:], rr[:], lof[:], Alu.add)
                    nc.vector.tensor_scalar(rr[:], rr[:], float(SENT), None,
                                            Alu.subtract)
                    nc.vector.tensor_tensor(rr[:], rr[:], valid_all[:], Alu.mult)
                    nc.vector.tensor_scalar(rec_all[:], rr[:], float(SENT), None,
                                            Alu.add)

                    if debug_outputs:
                        nc.sync.dma_start(dbg_slot[:], slot_all[:])
                        nc.sync.dma_start(dbg_rec[:], rec_all[:])

                    # ---- scatter my tokens into the [E,TPAD] table, then RS ----
                    ztab = sbt.tile([128, E * TPAD // 128], F32, tag="ztab")
                    nc.vector.memset(ztab[:], 0.0)
                    nc.sync.dma_start(
                        table_d[:].rearrange("(p n) -> p n", p=128), ztab[:])
                    table2d = table_d[:].rearrange("(n e) -> n e", e=1)
                    for i in range(NTT):
                        nc.gpsimd.indirect_dma_start(
                            table2d,
                            bass.IndirectOffsetOnAxis(
                                ap=slotidx[:, i:i + 1], axis=0),
                            tokid_sb[:, i:i + 1], None)
                    nc.gpsimd.collective_compute(
                        "ReduceScatter", Alu.add, replica_groups=RG,
                        ins=[table_d[:]], outs=[rstbl_d[:]])

                # ---- readback -> dispatch gather idx (wrapped via PE transp) ----
                tbl_nat = sbt.tile([128, NTT], F32, tag="tblnat")
                nc.sync.dma_start(
                    tbl_nat[:], rstbl_d[0:C].rearrange("(p n) -> p n", p=128))
                emp = sbt.tile([128, NTT], F32, tag="emp")
                nc.vector.tensor_scalar(emp[:], tbl_nat[:], 0.0, float(SENT + 1),
                                        Alu.is_equal, Alu.mult)
                nc.vector.tensor_tensor(tbl_nat[:], tbl_nat[:], emp[:], Alu.add)
                nc.vector.tensor_scalar(tbl_nat[:], tbl_nat[:], 1.0, None,
                                        Alu.subtract)
                if debug_outputs:
                    nc.sync.dma_start(dbg_tblidx[:], tbl_nat[:])
                gt_ps = pss.tile([16, 128], F32, tag="ptr", bufs=1)
                nc.tensor.transpose(gt_ps[:], tbl_nat[:], idn_sb[:])
                nc.vector.tensor_copy(gidx_rep[0:16, :], gt_ps[:])
                for rep in range(1, 8):
                    nc.sync.dma_start(gidx_rep[16 * rep:16 * (rep + 1), :],
                                      gidx_rep[0:16, :])

                # ---- recon gather idx (two PE transposes via DRAM bounce) ----
                rt_ps = pss.tile([16, 128], F32, tag="ptr", bufs=1)
                nc.tensor.transpose(rt_ps[:], rec_all[:], idn_sb[:])
                rT = sbt.tile([16, 128], F32, tag="rT")
                nc.vector.tensor_copy(rT[:], rt_ps[:])
                nc.sync.dma_start(
                    ridx_f_d[:].rearrange("(a b) -> a b", a=16), rT[:])
                rn = sbt.tile([128, NTT], F32, tag="rn")
                nc.sync.dma_start(
                    rn[:], ridx_f_d[:].rearrange("(a b) -> a b", a=128))
                rw_ps = pss.tile([16, 128], F32, tag="ptr", bufs=1)
                nc.tensor.transpose(rw_ps[:], rn[:], idn_sb[:])
                nc.vector.tensor_copy(ridx_rep[0:16, :], rw_ps[:])
                for rep in range(1, 8):
                    nc.sync.dma_start(ridx_rep[16 * rep:16 * (rep + 1), :],
                                      ridx_rep[0:16, :])

            # ===================== EXPERT FFN =====================
            with tc.tile_pool(name="f2", bufs=1) as f2, \
                 tc.tile_pool(name="fc", bufs=2) as fc:
                w2_sb = f2.tile([128, NJT, D], BF16)
                nc.sync.dma_start(w2_sb[:], w2[:])
                for ch in range(NCH):
                    xet = fc.tile([128, E, CHUNK], BF16, tag="xet")
                    nc.gpsimd.dma_gather(
                        xet[:], xbf[:], gidx_rep[:, ch * 16:(ch + 1) * 16],
                        CHUNK, CHUNK, D, transpose=True)

                    htf = fc.tile([128, NJT, CHUNK], BF16, tag="htf", bufs=1)
                    for jt in range(NJT):
                        ps = psb.tile([128, CHUNK], F32, tag="pbig")
                        for dt in range(E):
                            nc.tensor.matmul(
                                ps[:], w1_sb[:, dt, jt * 128:(jt + 1) * 128],
                                xet[:, dt, :], start=(dt == 0), stop=(dt == E - 1))
                        nc.scalar.activation(htf[:, jt, :], ps[:], Act.Relu,
                                             bias=b1_sb[:, jt:jt + 1], scale=1.0)

                    for ct in range(CHUNK // 128):
                        oe = fc.tile([128, D], BF16, tag="oe")
                        for nt in range(D // 512):
                            ps = psb.tile([128, 512], F32, tag="pbig")
                            for jt in range(NJT):
                                nc.tensor.matmul(
                                    ps[:],
                                    htf[:, jt, ct * 128:(ct + 1) * 128],
                                    w2_sb[:, jt, nt * 512:(nt + 1) * 512],
                                    start=(jt == 0), stop=(jt == NJT - 1))
                            nc.vector.tensor_tensor(
                                oe[:, nt * 512:(nt + 1) * 512], ps[:],
                                b2_rep[:, nt * 512:(nt + 1) * 512], Alu.add)
                        row0 = ch * CHUNK + ct * 128
                        nc.sync.dma_start(agin_d[row0:row0 + 128, :], oe[:])

                    nc.gpsimd.collective_compute(
                        "AllGather", Alu.bypass, replica_groups=RG,
                        ins=[agin_d[ch * AGCH:(ch + 1) * AGCH, :]],
                        outs=[oe_all_d[ch * AGCH * NC:(ch + 1) * AGCH * NC, :]])

                # ---------- reconstruct my token shard (8 pieces) ----------
                for q in range(8):
                    rec = fc.tile([128, 2, D], BF16, tag="rec")
                    nc.gpsimd.dma_gather(
                        rec[:], oe_all_d[:], ridx_rep[:, q * 16:(q + 1) * 16],
                        256, 256, D, transpose=False)
                    for i in range(2):
                        tt = q * 2 + i
                        of = fc.tile([128, D], F32, tag="of")
                        nc.vector.tensor_scalar(of[:], rec[:, i, :],
                                                gate_all[:, tt:tt + 1], None,
                                                Alu.mult)
                        nc.sync.dma_start(out[tt * 128:(tt + 1) * 128, :], of[:])

    nc.compile()
    return nc


# ---------------------------------------------------------------------------
# host side
# ---------------------------------------------------------------------------
def _to_bf16(a: np.ndarray) -> np.ndarray:
    import jax
    import jax.numpy as jnp
    with jax.default_device(jax.devices("cpu")[0]):
        return np.asarray(jnp.asarray(a, jnp.bfloat16))


_NC_CACHE = {}


def _get_nc(debug_outputs=DEBUG_OUTPUTS):
    if debug_outputs not in _NC_CACHE:
        _NC_CACHE[debug_outputs] = build(debug_outputs)
    return _NC_CACHE[debug_outputs]


def prepare_in_maps(x, wr1, br1, wr2, br2, w1, b1, w2, b2):
    x = np.asarray(x, np.float32)
    wr1 = np.asarray(wr1, np.float32)
    wr2 = np.asarray(wr2, np.float32)
    br1 = np.asarray(br1, np.float32)
    br2 = np.asarray(br2, np.float32)
    w1 = np.asarray(w1, np.float32)
    w2 = np.asarray(w2, np.float32)
    b1 = np.asarray(b1, np.float32)
    b2 = np.asarray(b2, np.float32)

    xpad = np.zeros((T + 1, D), np.float32)
    xpad[:T] = x
    xbf = _to_bf16(xpad)
    xbl = _to_bf16(xpad - xbf.astype(np.float32))

    iota8 = np.tile(np.arange(E, dtype=np.float32), (128, 1))
    ut = np.triu(np.ones((128, 128), np.float32))
    idn = np.eye(128, dtype=np.float32)
    spm = np.kron(np.triu(np.ones((NTT, NTT), np.float32), 1),
                  np.eye(E, dtype=np.float32))

    wr1p = np.ascontiguousarray(wr1.reshape(E, 128, D).transpose(1, 0, 2))
    wr1h_ = _to_bf16(wr1p)
    base = dict(
        wr1h=wr1h_, wr1l=_to_bf16(wr1p - wr1h_.astype(np.float32)),
        wr2=np.ascontiguousarray(wr2.reshape(E, 128, E).transpose(1, 0, 2)),
        br1=np.ascontiguousarray(br1.reshape(E, 128).T),
        br2=br2.reshape(1, E),
        xbf=xbf, xbl=xbl, UT=ut, SPM=spm, IOTA8=iota8, IDN=idn,
    )
    maps = []
    for k in range(NC):
        m = dict(base)
        xs = x[k * TS:(k + 1) * TS]                      # [2048, 1024]
        xt = np.ascontiguousarray(
            xs.T.reshape(E, 128, TS).transpose(1, 0, 2)
            .reshape(128, E, NTC, 512).transpose(2, 0, 1, 3))
        m["xTh"] = _to_bf16(xt)
        m["w1"] = _to_bf16(np.ascontiguousarray(
            w1[k].reshape(E, 128, H).transpose(1, 0, 2)))
        m["w2"] = _to_bf16(np.ascontiguousarray(
            w2[k].reshape(NJT, 128, D).transpose(1, 0, 2)))
        m["b1"] = np.ascontiguousarray(b1[k].reshape(NJT, 128).T)
        m["b2"] = b2[k].reshape(1, D)
        mask = np.zeros((E, 128), np.float32)
        mask[:k, :] = 1.0
        m["MASK"] = mask
        tokid = np.zeros((128, NTT), np.float32)
        tl = np.arange(TS)
        tokid[tl % 128, tl // 128] = k * TS + tl + 1.0
        m["TOKID"] = tokid
        m["KOFF"] = np.array([[k * TS]], np.float32)
        maps.append(m)
    return maps


def run(inputs, trace=False, debug_outputs=DEBUG_OUTPUTS, **kw):
    nc = _get_nc(debug_outputs)
    in_maps = prepare_in_maps(**inputs)
    return bass_utils.run_bass_kernel_spmd(
        nc, in_maps, core_ids=list(range(NC)), trace=trace, **kw)


def kernel(**inputs) -> np.ndarray:
    res = run(inputs)
    return np.concatenate([res.results[k]["out"] for k in range(NC)], axis=0)
